# revision 30
# baseline (speedup 1.0000x reference)
"""Causal self-attention (B=8, T=2048, C=128, H=4, D=32) on 8 trn2 NeuronCores.

Sharding: data-parallel over batch — core b handles batch element b.

Per-core algorithm (PE matmuls in fp32r = full-rate rounded fp32, except the
PV stage in bf16 so its head pair can col-tile across the PE array):
  xT = transpose(x)                      # PE transposes, [C, T]
  qT, kT = (x @ Wq|k + b)^T              # weights stationary, out [C,T] chunks
  v   = x @ Wv + bv                      # natural [T, C], packed into vaug
  vaug[tk-tile a] = [v_h | 1 | 0...]     # [128, 64] per head: the ones column
                                         # accumulates the softmax denominator
  flat software pipeline over (tq-block j, head pair, tk-tile a <= 4j+3):
      S^T[tk,tq] = kT_h.T @ qT_h         # K=32 row-packed pairs, PSUM [128,1024]
      (diag a: += lower-tri -30000 mask)
      E = exp(S * 1/sqrt(32))            # ACT, fused scale, bf16 out
      psum_y += vaug_a.T @ E             # col-packed pairs, M=64; row 32 = sum E
  per (j, pair): reciprocal of denominator rows, DMA broadcast via DRAM
  scratch, one multiply -> ynorm; projection accumulated per pair into one
  [128, 512] PSUM tile; one bias add + one DMA out per j.
"""

import sys

sys.path.insert(0, "/opt/trn_rl_repo")

import numpy as np

B, T, C = 8, 2048, 128
H, D = 4, 32
N_CORES = 8
TQ = 512          # tq block
NT = T // 128     # 16 tk tiles
NJ = T // TQ      # 4 tq blocks
SCALE = 1.0 / np.sqrt(D)
MASKVAL = -30000.0

_cache = {}


def _build():
    import concourse.bass as bass
    import concourse.mybir as mybir
    import concourse.tile as tile
    from concourse import bacc
    from concourse.masks import make_identity

    dt = mybir.dt
    AF = mybir.ActivationFunctionType
    nc = bacc.Bacc()

    x = nc.dram_tensor("x", [T, C], dt.float32, kind="ExternalInput")
    w_qkv = nc.dram_tensor("w_qkv", [C, 3 * C], dt.float32, kind="ExternalInput")
    b_qkv = nc.dram_tensor("b_qkv", [3 * C], dt.float32, kind="ExternalInput")
    w_proj = nc.dram_tensor("w_proj", [C, C], dt.float32, kind="ExternalInput")
    b_proj = nc.dram_tensor("b_proj", [C], dt.float32, kind="ExternalInput")
    y = nc.dram_tensor("y", [T, C], dt.float32, kind="ExternalOutput")
    # DRAM scratch for the denominator-reciprocal partition broadcast
    rscr = nc.dram_tensor("rscr", [NJ, 2, 2, TQ], dt.float32, kind="Internal")

    with tile.TileContext(nc) as tc:
        with (
            nc.allow_low_precision(reason="fp32r/bf16 matmuls; validated vs ref"),
            tc.tile_pool(name="const", bufs=1) as const,
            tc.tile_pool(name="big", bufs=1) as big,
            tc.tile_pool(name="sb", bufs=4) as sb,
            tc.tile_pool(name="esb", bufs=4) as esb,
            tc.tile_pool(name="ysb", bufs=3) as ysb,
            tc.tile_pool(name="ps_misc", bufs=2, space="PSUM") as ps_misc,
            tc.tile_pool(name="ps_s", bufs=2, space="PSUM") as ps_s,
            tc.tile_pool(name="ps_y", bufs=2, space="PSUM") as ps_y,
        ):
            # ---------------- critical-path constants ----------------
            ident = const.tile([128, 128], dt.float32)
            make_identity(nc, ident)

            # lower-triangle causal mask for S^T diag tiles, duplicated 2x so
            # both head-halves mask in one DVE op.  masked iff tk > tq i.e.
            # partition p > free f:  keep when (f - p) >= 0.
            trimask = const.tile([128, 2, 128], dt.float32)
            nc.gpsimd.memset(trimask, 0.0)
            for half in range(2):
                nc.gpsimd.affine_select(
                    out=trimask[:, half, :],
                    in_=trimask[:, half, :],
                    compare_op=mybir.AluOpType.is_ge,
                    fill=MASKVAL,
                    base=0,
                    pattern=[[1, 128]],
                    channel_multiplier=-1,
                )

            # dummy exp so the ACT table set loads while QKV runs
            dumm = const.tile([1, 1], dt.float32)
            nc.scalar.activation(dumm, trimask[0:1, 0, 0:1], AF.Exp)

            # biases: b_q/b_k as [128,1] per-partition columns
            bqk = const.tile([128, 2], dt.float32)
            nc.sync.dma_start(
                out=bqk, in_=b_qkv[0:256].rearrange("(j p) -> p j", p=128)
            )

            # w_qkv rounded to fp32r (first DVE op: q/k path is the head of
            # the pipeline)
            w_sb = const.tile([128, 3 * C], dt.float32)
            nc.sync.dma_start(out=w_sb, in_=w_qkv[:, :])
            w_r = const.tile([128, 3 * C], dt.float32r)
            nc.vector.tensor_copy(w_r, w_sb)

            # persistent activations
            xT = big.tile([128, T], dt.float32r)       # [c, t]
            qkT = big.tile([128, 2, T], dt.float32r)   # [c, {q,k}, t]
            # vaug layout per tk-tile a: [128, 4 heads, 64]; head block =
            # [v_h (32) | 1.0 | zeros(31)]
            vaug = big.tile([128, NT, 4, 64], dt.bfloat16)

            def emit_x1(a):
                x_t = sb.tile([128, 128], dt.float32, tag="xin")
                nc.sync.dma_start(out=x_t, in_=x[128 * a:128 * (a + 1), :])
                p_tr = ps_misc.tile([128, 128], dt.float32, tag="misc")
                nc.tensor.transpose(p_tr, x_t, ident)
                nc.vector.tensor_copy(xT[:, 128 * a:128 * (a + 1)], p_tr)

            def emit_qk1(g, ch):
                p_qk = ps_misc.tile([128, TQ], dt.float32, tag="misc")
                nc.tensor.matmul(
                    p_qk,
                    w_r[:, 128 * ch:128 * (ch + 1)],
                    xT[:, TQ * g:TQ * (g + 1)],
                    start=True, stop=True,
                )
                nc.vector.tensor_scalar_add(
                    qkT[:, ch, TQ * g:TQ * (g + 1)], p_qk, bqk[:, ch:ch + 1]
                )

            def emit_v1(a):
                p_v = ps_misc.tile([128, 128], dt.float32, tag="misc")
                nc.tensor.matmul(
                    p_v,
                    xT[:, 128 * a:128 * (a + 1)],
                    w_r[:, 256:384],
                    start=True, stop=True,
                )
                nc.vector.tensor_add(
                    vaug[:, a, :, 0:32],
                    p_v.rearrange("p (h d) -> p h d", h=4),
                    bvb.rearrange("p (h d) -> p h d", h=4),
                )

            def emit_xqk(g):
                for a in range(4 * g, 4 * g + 4):
                    emit_x1(a)
                emit_qk1(g, 0)
                emit_qk1(g, 1)

            def emit_v(g):
                for a in range(4 * g, 4 * g + 4):
                    emit_v1(a)

            emit_xqk(0)

            # ---------------- remaining constants ----------------
            # w_proj split into two "pair" tiles matching the pair layout of
            # the PV output (head A rows 0-31, denominator row 32, zeros,
            # head B rows 64-95, ...).  Rows 32-63/96-127 must be zero so the
            # r*(1/r)=1 rows and zero rows contribute nothing.
            wp_pair = []
            for pair in range(2):
                wp_sb = const.tile([128, C], dt.float32, name=f"wp_sb_{pair}")
                nc.vector.memset(wp_sb, 0.0)
                nc.sync.dma_start(
                    out=wp_sb[0:32, :], in_=w_proj[64 * pair:64 * pair + 32, :]
                )
                nc.sync.dma_start(
                    out=wp_sb[64:96, :], in_=w_proj[64 * pair + 32:64 * pair + 64, :]
                )
                wp_r = const.tile([128, C], dt.float32r, name=f"wp_r_{pair}")
                nc.vector.tensor_copy(wp_r, wp_sb)
                wp_pair.append(wp_r)

            # broadcast tiles for free-dim biases (b_v, b_proj): row vector in
            # one partition, K=1 matmul against ones -> [128, 128] all rows.
            brow = const.tile([1, 256], dt.float32)
            nc.sync.dma_start(out=brow[:, 0:128], in_=b_qkv[256:384][None, :])
            nc.sync.dma_start(out=brow[:, 128:256], in_=b_proj[:][None, :])
            brow_r = const.tile([1, 256], dt.float32r)
            nc.vector.tensor_copy(brow_r, brow)
            ones1_f = const.tile([1, 128], dt.float32)
            nc.vector.memset(ones1_f, 1.0)
            ones1 = const.tile([1, 128], dt.float32r)
            nc.vector.tensor_copy(ones1, ones1_f)
            p_b = ps_misc.tile([128, 256], dt.float32, tag="misc")
            nc.tensor.matmul(p_b, ones1, brow_r, start=True, stop=True)
            bvb = const.tile([128, 128], dt.float32)    # b_v broadcast
            bpb4 = const.tile([128, 4, 128], dt.float32)  # b_proj bcast x4
            nc.vector.tensor_copy(bvb, p_b[:, 0:128])
            for m in range(4):
                nc.vector.tensor_copy(bpb4[:, m, :], p_b[:, 128:256])

            nc.vector.memset(vaug, 0.0)
            nc.vector.memset(vaug[:, :, :, 32:33], 1.0)

            emit_v(0)
            emit_xqk(1)
            emit_v(1)

            # ---------------- attention pipeline ----------------
            p_os = {}
            ynorms_d = {}

            def emit_pv(pend):
                p_yp, jp, pairp, e_p, a_p, off_p = pend
                for ih in range(2):
                    nc.tensor.matmul(
                        p_yp[64 * ih:64 * (ih + 1), off_p:],
                        vaug[:, a_p, 2 * pairp + ih, :],
                        e_p[:, TQ * ih + off_p:TQ * (ih + 1)],
                        start=(a_p == 0), stop=(a_p == 4 * jp + 3),
                        tile_position=(0, 64 * ih),
                    )

            def emit_norm(p_y, j, pair):
                # reciprocal straight off the PV PSUM tile (only rows 32/96 =
                # denominators matter), DMA-broadcast those rows across
                # partitions 0-63 / 64-127 via DRAM, one elementwise mult.
                rrec = ysb.tile([128, TQ], dt.float32, tag="rrec",
                                name=f"rrec_{j}_{pair}")
                nc.vector.reciprocal(rrec, p_y)
                rb = ysb.tile([128, TQ], dt.float32, tag="rb",
                              name=f"rb_{j}_{pair}")
                for half in range(2):
                    nc.sync.dma_start(
                        out=rscr[j, pair, half][None, :],
                        in_=rrec[32 + 64 * half:33 + 64 * half, :],
                    )
                    src = rscr[j, pair, half]
                    nc.sync.dma_start(
                        out=rb[64 * half:64 * (half + 1), :],
                        in_=bass.AP(
                            tensor=src.tensor,
                            offset=src.offset,
                            ap=[[0, 64], [1, TQ]],
                        ),
                    )
                ynorm = ysb.tile([128, TQ], dt.float32r, tag="ynorm",
                                 name=f"ynorm_{j}_{pair}", bufs=3)
                nc.vector.tensor_mul(ynorm, p_y, rb)
                return ynorm

            def emit_proj_m(j, m):
                p_o = ps_misc.tile([128, 128], dt.float32, tag="misc")
                for pr in range(2):
                    nc.tensor.matmul(
                        p_o,
                        ynorms_d[j][pr][:, 128 * m:128 * (m + 1)],
                        wp_pair[pr],
                        start=(pr == 0), stop=(pr == 1),
                    )
                o_t = sb.tile([128, 128], dt.float32, tag="out")
                nc.vector.tensor_add(o_t, p_o, bpb4[:, 0, :])
                t0 = TQ * j + 128 * m
                nc.sync.dma_start(out=y[t0:t0 + 128, :], in_=o_t)

            # flat software pipeline over all (j, pair, a) tiles: the PV
            # matmuls trail the S/exp stream by one item so the PE queue
            # always has independent S work ahead of each exp dependency;
            # group-boundary work (normalization, projection halves, QKV for
            # j+2) lands behind the next group's S matmuls.
            items = [
                (j, pair, a)
                for j in range(NJ)
                for a in range(4 * j + 4)
                for pair in range(2)
            ]
            p_ys = {}
            pend = None

            from collections import deque
            bg = deque()

            def flush(pend):
                emit_pv(pend)
                _, jp, pairp, _, a_p, _ = pend
                if a_p == 4 * jp + 3:  # group (jp, pairp) complete
                    ynorm = emit_norm(p_ys.pop((jp, pairp)), jp, pairp)
                    ynorms_d.setdefault(jp, []).append(ynorm)
                    if pairp == 1:
                        if jp + 2 < NJ:
                            g = jp + 2
                            for aa in range(4 * g, 4 * g + 4):
                                bg.append(lambda aa=aa: emit_x1(aa))
                            bg.append(lambda g=g: emit_qk1(g, 0))
                            bg.append(lambda g=g: emit_qk1(g, 1))
                            for aa in range(4 * g, 4 * g + 4):
                                bg.append(lambda aa=aa: emit_v1(aa))
                        for m in range(TQ // 128):
                            bg.append(lambda jp=jp, m=m: emit_proj_m(jp, m))
                if bg:
                    bg.popleft()()

            for j, pair, a in items:
                if a == 0:
                    p_ys[(j, pair)] = ps_y.tile(
                        [128, TQ], dt.float32, tag="py", name=f"p_y_{j}_{pair}"
                    )
                p_y = p_ys[(j, pair)]
                r = a - 4 * j
                off = 128 * r if r > 0 else 0
                p_s = ps_s.tile([128, 1024], dt.float32, tag="s")
                for ih, h in enumerate((2 * pair, 2 * pair + 1)):
                    nc.tensor.matmul(
                        p_s[:, TQ * ih + off:TQ * (ih + 1)],
                        qkT[32 * h:32 * (h + 1), 1, 128 * a:128 * (a + 1)],
                        qkT[32 * h:32 * (h + 1), 0, TQ * j + off:TQ * (j + 1)],
                        start=True, stop=True,
                        tile_position=(32 * h, 0),
                    )
                e_t = esb.tile([128, 1024], dt.bfloat16, tag="e")
                nc.scalar.activation(
                    e_t.rearrange("p (i f) -> p i f", i=2)[:, :, off:],
                    p_s.rearrange("p (i f) -> p i f", i=2)[:, :, off:],
                    AF.Exp,
                    scale=float(SCALE),
                )
                if r >= 0:
                    # diag tile: zero the causal triangle of the exp'd tile
                    # (tk > tq, i.e. partition p > local col f) on GPSIMD so
                    # the DVE stays off the S->exp->PV critical path
                    sel = e_t.rearrange("p (i f) -> p i f", i=2)[:, :, off:off + 128]
                    nc.gpsimd.affine_select(
                        out=sel,
                        in_=sel,
                        compare_op=mybir.AluOpType.is_ge,
                        fill=0.0,
                        base=0,
                        pattern=[[0, 2], [1, 128]],
                        channel_multiplier=-1,
                    )
                if pend is not None:
                    flush(pend)
                pend = (p_y, j, pair, e_t, a, off)
            flush(pend)
            while bg:
                bg.popleft()()

    nc.compile()
    return nc


def _get_nc():
    if "nc" not in _cache:
        _cache["nc"] = _build()
    return _cache["nc"]


def run(inputs, trace=False):
    from concourse.bass_utils import run_bass_kernel_spmd

    nc = _get_nc()
    x = np.asarray(inputs["x"], dtype=np.float32)
    w_qkv = np.ascontiguousarray(np.asarray(inputs["w_qkv"], dtype=np.float32))
    b_qkv = np.ascontiguousarray(np.asarray(inputs["b_qkv"], dtype=np.float32))
    w_proj = np.ascontiguousarray(np.asarray(inputs["w_proj"], dtype=np.float32))
    b_proj = np.ascontiguousarray(np.asarray(inputs["b_proj"], dtype=np.float32))
    in_maps = [
        {
            "x": np.ascontiguousarray(x[b]),
            "w_qkv": w_qkv,
            "b_qkv": b_qkv,
            "w_proj": w_proj,
            "b_proj": b_proj,
        }
        for b in range(N_CORES)
    ]
    res = run_bass_kernel_spmd(
        nc, in_maps, core_ids=list(range(N_CORES)), trace=trace
    )
    out = np.stack([res.results[b]["y"] for b in range(N_CORES)], axis=0)
    return out, res


def kernel(**inputs) -> np.ndarray:
    out, _ = run(inputs, trace=False)
    return out


# revision 31
# speedup vs baseline: 1.0763x; 1.0763x over previous
"""Causal self-attention (B=8, T=2048, C=128, H=4, D=32) on 8 trn2 NeuronCores.

Sharding: data-parallel over batch — core b handles batch element b.

Per-core algorithm (PE matmuls in fp32r = full-rate rounded fp32, except the
PV stage in bf16 so its head pair can col-tile across the PE array):
  xT = transpose(x)                      # PE transposes, [C, T]
  qT, kT = (x @ Wq|k + b)^T              # weights stationary, out [C,T] chunks
  v   = x @ Wv + bv                      # natural [T, C], packed into vaug
  vaug[tk-tile a] = [v_h | 1 | 0...]     # [128, 64] per head: the ones column
                                         # accumulates the softmax denominator
  flat software pipeline over (tq-block j, head pair, tk-tile a <= 4j+3):
      S^T[tk,tq] = kT_h.T @ qT_h         # K=32 row-packed pairs, PSUM [128,1024]
      (diag a: += lower-tri -30000 mask)
      E = exp(S * 1/sqrt(32))            # ACT, fused scale, bf16 out
      psum_y += vaug_a.T @ E             # col-packed pairs, M=64; row 32 = sum E
  per (j, pair): reciprocal of denominator rows, DMA broadcast via DRAM
  scratch, one multiply -> ynorm; projection accumulated per pair into one
  [128, 512] PSUM tile; one bias add + one DMA out per j.
"""

import sys

sys.path.insert(0, "/opt/trn_rl_repo")

import numpy as np

B, T, C = 8, 2048, 128
H, D = 4, 32
N_CORES = 8
TQ = 512          # tq block
NT = T // 128     # 16 tk tiles
NJ = T // TQ      # 4 tq blocks
SCALE = 1.0 / np.sqrt(D)
MASKVAL = -30000.0

_cache = {}


def _build():
    import concourse.bass as bass
    import concourse.mybir as mybir
    import concourse.tile as tile
    from concourse import bacc
    from concourse.masks import make_identity

    dt = mybir.dt
    AF = mybir.ActivationFunctionType
    nc = bacc.Bacc()

    x = nc.dram_tensor("x", [T, C], dt.float32, kind="ExternalInput")
    w_qkv = nc.dram_tensor("w_qkv", [C, 3 * C], dt.float32, kind="ExternalInput")
    b_qkv = nc.dram_tensor("b_qkv", [3 * C], dt.float32, kind="ExternalInput")
    w_proj = nc.dram_tensor("w_proj", [C, C], dt.float32, kind="ExternalInput")
    b_proj = nc.dram_tensor("b_proj", [C], dt.float32, kind="ExternalInput")
    y = nc.dram_tensor("y", [T, C], dt.float32, kind="ExternalOutput")
    # DRAM scratch for the denominator-reciprocal partition broadcast
    rscr = nc.dram_tensor("rscr", [NJ, 2, 2, TQ], dt.float32, kind="Internal")

    with tile.TileContext(nc) as tc:
        with (
            nc.allow_low_precision(reason="fp32r/bf16 matmuls; validated vs ref"),
            tc.tile_pool(name="const", bufs=1) as const,
            tc.tile_pool(name="big", bufs=1) as big,
            tc.tile_pool(name="sb", bufs=4) as sb,
            tc.tile_pool(name="esb", bufs=4) as esb,
            tc.tile_pool(name="ysb", bufs=3) as ysb,
            tc.tile_pool(name="ps_misc", bufs=2, space="PSUM") as ps_misc,
            tc.tile_pool(name="ps_s", bufs=2, space="PSUM") as ps_s,
            tc.tile_pool(name="ps_y", bufs=2, space="PSUM") as ps_y,
        ):
            # ---------------- critical-path constants ----------------
            ident = const.tile([128, 128], dt.float32)
            make_identity(nc, ident)

            # lower-triangle causal mask for S^T diag tiles, duplicated 2x so
            # both head-halves mask in one DVE op.  masked iff tk > tq i.e.
            # partition p > free f:  keep when (f - p) >= 0.
            trimask = const.tile([128, 2, 128], dt.float32)
            nc.gpsimd.memset(trimask, 0.0)
            for half in range(2):
                nc.gpsimd.affine_select(
                    out=trimask[:, half, :],
                    in_=trimask[:, half, :],
                    compare_op=mybir.AluOpType.is_ge,
                    fill=MASKVAL,
                    base=0,
                    pattern=[[1, 128]],
                    channel_multiplier=-1,
                )

            # dummy exp so the ACT table set loads while QKV runs
            dumm = const.tile([1, 1], dt.float32)
            nc.scalar.activation(dumm, trimask[0:1, 0, 0:1], AF.Exp)

            # biases: b_q/b_k as [128,1] per-partition columns
            bqk = const.tile([128, 2], dt.float32)
            nc.sync.dma_start(
                out=bqk, in_=b_qkv[0:256].rearrange("(j p) -> p j", p=128)
            )

            # w_qkv rounded to fp32r (first DVE op: q/k path is the head of
            # the pipeline)
            w_sb = const.tile([128, 3 * C], dt.float32)
            nc.sync.dma_start(out=w_sb, in_=w_qkv[:, :])
            w_r = const.tile([128, 3 * C], dt.float32r)
            nc.vector.tensor_copy(w_r, w_sb)

            # persistent activations
            xT = big.tile([128, T], dt.float32r)       # [c, t]
            qkT = big.tile([128, 2, T], dt.float32r)   # [c, {q,k}, t]
            # vaug layout per tk-tile a: [128, 4 heads, 64]; head block =
            # [v_h (32) | 1.0 | zeros(31)]
            vaug = big.tile([128, NT, 4, 64], dt.bfloat16)

            def emit_x1(a):
                x_t = sb.tile([128, 128], dt.float32, tag="xin")
                nc.sync.dma_start(out=x_t, in_=x[128 * a:128 * (a + 1), :])
                p_tr = ps_misc.tile([128, 128], dt.float32, tag="misc")
                nc.tensor.transpose(p_tr, x_t, ident)
                nc.vector.tensor_copy(xT[:, 128 * a:128 * (a + 1)], p_tr)

            def emit_qk1(g, ch):
                p_qk = ps_misc.tile([128, TQ], dt.float32, tag="misc")
                nc.tensor.matmul(
                    p_qk,
                    w_r[:, 128 * ch:128 * (ch + 1)],
                    xT[:, TQ * g:TQ * (g + 1)],
                    start=True, stop=True,
                )
                nc.vector.tensor_scalar_add(
                    qkT[:, ch, TQ * g:TQ * (g + 1)], p_qk, bqk[:, ch:ch + 1]
                )

            def emit_v1(a):
                p_v = ps_misc.tile([128, 128], dt.float32, tag="misc")
                nc.tensor.matmul(
                    p_v,
                    xT[:, 128 * a:128 * (a + 1)],
                    w_r[:, 256:384],
                    start=True, stop=True,
                )
                nc.vector.tensor_add(
                    vaug[:, a, :, 0:32],
                    p_v.rearrange("p (h d) -> p h d", h=4),
                    bvb.rearrange("p (h d) -> p h d", h=4),
                )

            def emit_xqk(g):
                for a in range(4 * g, 4 * g + 4):
                    emit_x1(a)
                emit_qk1(g, 0)
                emit_qk1(g, 1)

            def emit_v(g):
                for a in range(4 * g, 4 * g + 4):
                    emit_v1(a)

            emit_xqk(0)

            # ---------------- remaining constants ----------------
            # w_proj split into two "pair" tiles matching the pair layout of
            # the PV output (head A rows 0-31, denominator row 32, zeros,
            # head B rows 64-95, ...).  Rows 32-63/96-127 must be zero so the
            # r*(1/r)=1 rows and zero rows contribute nothing.
            wp_pair = []
            for pair in range(2):
                wp_sb = const.tile([128, C], dt.float32, name=f"wp_sb_{pair}")
                nc.vector.memset(wp_sb, 0.0)
                nc.sync.dma_start(
                    out=wp_sb[0:32, :], in_=w_proj[64 * pair:64 * pair + 32, :]
                )
                nc.sync.dma_start(
                    out=wp_sb[64:96, :], in_=w_proj[64 * pair + 32:64 * pair + 64, :]
                )
                wp_r = const.tile([128, C], dt.float32r, name=f"wp_r_{pair}")
                nc.vector.tensor_copy(wp_r, wp_sb)
                wp_pair.append(wp_r)

            # broadcast tiles for free-dim biases (b_v, b_proj): row vector in
            # one partition, K=1 matmul against ones -> [128, 128] all rows.
            brow = const.tile([1, 256], dt.float32)
            nc.sync.dma_start(out=brow[:, 0:128], in_=b_qkv[256:384][None, :])
            nc.sync.dma_start(out=brow[:, 128:256], in_=b_proj[:][None, :])
            brow_r = const.tile([1, 256], dt.float32r)
            nc.vector.tensor_copy(brow_r, brow)
            ones1_f = const.tile([1, 128], dt.float32)
            nc.vector.memset(ones1_f, 1.0)
            ones1 = const.tile([1, 128], dt.float32r)
            nc.vector.tensor_copy(ones1, ones1_f)
            p_b = ps_misc.tile([128, 256], dt.float32, tag="misc")
            nc.tensor.matmul(p_b, ones1, brow_r, start=True, stop=True)
            bvb = const.tile([128, 128], dt.float32)    # b_v broadcast
            bpb4 = const.tile([128, 4, 128], dt.float32)  # b_proj bcast x4
            nc.vector.tensor_copy(bvb, p_b[:, 0:128])
            for m in range(4):
                nc.vector.tensor_copy(bpb4[:, m, :], p_b[:, 128:256])

            nc.vector.memset(vaug, 0.0)
            nc.vector.memset(vaug[:, :, :, 32:33], 1.0)

            emit_v(0)
            emit_xqk(1)
            emit_v(1)

            # ---------------- attention pipeline ----------------
            p_os = {}
            ynorms_d = {}

            def emit_pv(pend):
                p_yp, jp, pairp, e_p, a_p, off_p = pend
                for ih in range(2):
                    nc.tensor.matmul(
                        p_yp[64 * ih:64 * (ih + 1), off_p:],
                        vaug[:, a_p, 2 * pairp + ih, :],
                        e_p[:, TQ * ih + off_p:TQ * (ih + 1)],
                        start=(a_p == 0), stop=(a_p == 4 * jp + 3),
                        tile_position=(0, 64 * ih),
                    )

            def emit_norm(p_y, j, pair):
                # reciprocal straight off the PV PSUM tile (only rows 32/96 =
                # denominators matter), DMA-broadcast those rows across
                # partitions 0-63 / 64-127 via DRAM, one elementwise mult.
                rrec = ysb.tile([128, TQ], dt.float32, tag="rrec",
                                name=f"rrec_{j}_{pair}")
                nc.vector.reciprocal(rrec, p_y)
                rb = ysb.tile([128, TQ], dt.float32, tag="rb",
                              name=f"rb_{j}_{pair}")
                for half in range(2):
                    nc.sync.dma_start(
                        out=rscr[j, pair, half][None, :],
                        in_=rrec[32 + 64 * half:33 + 64 * half, :],
                    )
                    src = rscr[j, pair, half]
                    nc.sync.dma_start(
                        out=rb[64 * half:64 * (half + 1), :],
                        in_=bass.AP(
                            tensor=src.tensor,
                            offset=src.offset,
                            ap=[[0, 64], [1, TQ]],
                        ),
                    )
                ynorm = ysb.tile([128, TQ], dt.float32r, tag="ynorm",
                                 name=f"ynorm_{j}_{pair}", bufs=3)
                nc.vector.tensor_mul(ynorm, p_y, rb)
                return ynorm

            def emit_proj_m(j, m):
                p_o = ps_misc.tile([128, 128], dt.float32, tag="misc")
                for pr in range(2):
                    nc.tensor.matmul(
                        p_o,
                        ynorms_d[j][pr][:, 128 * m:128 * (m + 1)],
                        wp_pair[pr],
                        start=(pr == 0), stop=(pr == 1),
                    )
                o_t = sb.tile([128, 128], dt.float32, tag="out")
                nc.vector.tensor_add(o_t, p_o, bpb4[:, 0, :])
                t0 = TQ * j + 128 * m
                nc.sync.dma_start(out=y[t0:t0 + 128, :], in_=o_t)

            # flat software pipeline over all (j, pair, a) tiles: the PV
            # matmuls trail the S/exp stream by one item so the PE queue
            # always has independent S work ahead of each exp dependency;
            # group-boundary work (normalization, projection halves, QKV for
            # j+2) lands behind the next group's S matmuls.
            items = [
                (j, pair, a)
                for j in range(NJ)
                for pair in range(2)
                for a in range(4 * j + 4)
            ]
            p_ys = {}
            pend = None

            from collections import deque
            bg = deque()

            def flush(pend):
                emit_pv(pend)
                _, jp, pairp, _, a_p, _ = pend
                if a_p == 4 * jp + 3:  # group (jp, pairp) complete
                    ynorm = emit_norm(p_ys.pop((jp, pairp)), jp, pairp)
                    ynorms_d.setdefault(jp, []).append(ynorm)
                    if pairp == 1:
                        if jp + 2 < NJ:
                            g = jp + 2
                            for aa in range(4 * g, 4 * g + 4):
                                bg.append(lambda aa=aa: emit_x1(aa))
                            bg.append(lambda g=g: emit_qk1(g, 0))
                            bg.append(lambda g=g: emit_qk1(g, 1))
                            for aa in range(4 * g, 4 * g + 4):
                                bg.append(lambda aa=aa: emit_v1(aa))
                        for m in range(TQ // 128):
                            bg.append(lambda jp=jp, m=m: emit_proj_m(jp, m))
                if bg:
                    bg.popleft()()

            for j, pair, a in items:
                if a == 0:
                    p_ys[(j, pair)] = ps_y.tile(
                        [128, TQ], dt.float32, tag="py", name=f"p_y_{j}_{pair}"
                    )
                p_y = p_ys[(j, pair)]
                r = a - 4 * j
                off = 128 * r if r > 0 else 0
                p_s = ps_s.tile([128, 1024], dt.float32, tag="s")
                for ih, h in enumerate((2 * pair, 2 * pair + 1)):
                    nc.tensor.matmul(
                        p_s[:, TQ * ih + off:TQ * (ih + 1)],
                        qkT[32 * h:32 * (h + 1), 1, 128 * a:128 * (a + 1)],
                        qkT[32 * h:32 * (h + 1), 0, TQ * j + off:TQ * (j + 1)],
                        start=True, stop=True,
                        tile_position=(32 * h, 0),
                    )
                e_t = esb.tile([128, 1024], dt.bfloat16, tag="e")
                nc.scalar.activation(
                    e_t.rearrange("p (i f) -> p i f", i=2)[:, :, off:],
                    p_s.rearrange("p (i f) -> p i f", i=2)[:, :, off:],
                    AF.Exp,
                    scale=float(SCALE),
                )
                if r >= 0:
                    # diag tile: zero the causal triangle of the exp'd tile
                    # (tk > tq, i.e. partition p > local col f) on GPSIMD so
                    # the DVE stays off the S->exp->PV critical path
                    sel = e_t.rearrange("p (i f) -> p i f", i=2)[:, :, off:off + 128]
                    nc.gpsimd.affine_select(
                        out=sel,
                        in_=sel,
                        compare_op=mybir.AluOpType.is_ge,
                        fill=0.0,
                        base=0,
                        pattern=[[0, 2], [1, 128]],
                        channel_multiplier=-1,
                    )
                if pend is not None:
                    flush(pend)
                pend = (p_y, j, pair, e_t, a, off)
            flush(pend)
            while bg:
                bg.popleft()()

    nc.compile()
    return nc


def _get_nc():
    if "nc" not in _cache:
        _cache["nc"] = _build()
    return _cache["nc"]


def run(inputs, trace=False):
    from concourse.bass_utils import run_bass_kernel_spmd

    nc = _get_nc()
    x = np.asarray(inputs["x"], dtype=np.float32)
    w_qkv = np.ascontiguousarray(np.asarray(inputs["w_qkv"], dtype=np.float32))
    b_qkv = np.ascontiguousarray(np.asarray(inputs["b_qkv"], dtype=np.float32))
    w_proj = np.ascontiguousarray(np.asarray(inputs["w_proj"], dtype=np.float32))
    b_proj = np.ascontiguousarray(np.asarray(inputs["b_proj"], dtype=np.float32))
    in_maps = [
        {
            "x": np.ascontiguousarray(x[b]),
            "w_qkv": w_qkv,
            "b_qkv": b_qkv,
            "w_proj": w_proj,
            "b_proj": b_proj,
        }
        for b in range(N_CORES)
    ]
    res = run_bass_kernel_spmd(
        nc, in_maps, core_ids=list(range(N_CORES)), trace=trace
    )
    out = np.stack([res.results[b]["y"] for b in range(N_CORES)], axis=0)
    return out, res


def kernel(**inputs) -> np.ndarray:
    out, _ = run(inputs, trace=False)
    return out


# revision 32
# speedup vs baseline: 1.0813x; 1.0047x over previous
"""Causal self-attention (B=8, T=2048, C=128, H=4, D=32) on 8 trn2 NeuronCores.

Sharding: data-parallel over batch — core b handles batch element b.

Per-core algorithm (PE matmuls in fp32r = full-rate rounded fp32, except the
PV stage in bf16 so its head pair can col-tile across the PE array):
  xT = transpose(x)                      # PE transposes, [C, T]
  qT, kT = (x @ Wq|k + b)^T              # weights stationary, out [C,T] chunks
  v   = x @ Wv + bv                      # natural [T, C], packed into vaug
  vaug[tk-tile a] = [v_h | 1 | 0...]     # [128, 64] per head: the ones column
                                         # accumulates the softmax denominator
  flat software pipeline over (tq-block j, head pair, tk-tile a <= 4j+3):
      S^T[tk,tq] = kT_h.T @ qT_h         # K=32 row-packed pairs, PSUM [128,1024]
      (diag a: += lower-tri -30000 mask)
      E = exp(S * 1/sqrt(32))            # ACT, fused scale, bf16 out
      psum_y += vaug_a.T @ E             # col-packed pairs, M=64; row 32 = sum E
  per (j, pair): reciprocal of denominator rows, DMA broadcast via DRAM
  scratch, one multiply -> ynorm; projection accumulated per pair into one
  [128, 512] PSUM tile; one bias add + one DMA out per j.
"""

import sys

sys.path.insert(0, "/opt/trn_rl_repo")

import numpy as np

B, T, C = 8, 2048, 128
H, D = 4, 32
N_CORES = 8
TQ = 512          # tq block
NT = T // 128     # 16 tk tiles
NJ = T // TQ      # 4 tq blocks
SCALE = 1.0 / np.sqrt(D)
MASKVAL = -30000.0

_cache = {}


def _build():
    import concourse.bass as bass
    import concourse.mybir as mybir
    import concourse.tile as tile
    from concourse import bacc
    from concourse.masks import make_identity

    dt = mybir.dt
    AF = mybir.ActivationFunctionType
    nc = bacc.Bacc()

    x = nc.dram_tensor("x", [T, C], dt.float32, kind="ExternalInput")
    w_qkv = nc.dram_tensor("w_qkv", [C, 3 * C], dt.float32, kind="ExternalInput")
    b_qkv = nc.dram_tensor("b_qkv", [3 * C], dt.float32, kind="ExternalInput")
    w_proj = nc.dram_tensor("w_proj", [C, C], dt.float32, kind="ExternalInput")
    b_proj = nc.dram_tensor("b_proj", [C], dt.float32, kind="ExternalInput")
    y = nc.dram_tensor("y", [T, C], dt.float32, kind="ExternalOutput")
    # DRAM scratch for the denominator-reciprocal partition broadcast
    rscr = nc.dram_tensor("rscr", [NJ, 2, 2, TQ], dt.float32, kind="Internal")

    with tile.TileContext(nc) as tc:
        with (
            nc.allow_low_precision(reason="fp32r/bf16 matmuls; validated vs ref"),
            tc.tile_pool(name="const", bufs=1) as const,
            tc.tile_pool(name="big", bufs=1) as big,
            tc.tile_pool(name="sb", bufs=4) as sb,
            tc.tile_pool(name="esb", bufs=4) as esb,
            tc.tile_pool(name="ysb", bufs=3) as ysb,
            tc.tile_pool(name="ps_misc", bufs=2, space="PSUM") as ps_misc,
            tc.tile_pool(name="ps_s", bufs=2, space="PSUM") as ps_s,
            tc.tile_pool(name="ps_y", bufs=2, space="PSUM") as ps_y,
        ):
            # ---------------- critical-path constants ----------------
            ident = const.tile([128, 128], dt.float32)
            make_identity(nc, ident)

            # lower-triangle causal mask for S^T diag tiles, duplicated 2x so
            # both head-halves mask in one DVE op.  masked iff tk > tq i.e.
            # partition p > free f:  keep when (f - p) >= 0.
            trimask = const.tile([128, 2, 128], dt.float32)
            nc.gpsimd.memset(trimask, 0.0)
            for half in range(2):
                nc.gpsimd.affine_select(
                    out=trimask[:, half, :],
                    in_=trimask[:, half, :],
                    compare_op=mybir.AluOpType.is_ge,
                    fill=MASKVAL,
                    base=0,
                    pattern=[[1, 128]],
                    channel_multiplier=-1,
                )

            # dummy exp so the ACT table set loads while QKV runs
            dumm = const.tile([1, 1], dt.float32)
            nc.scalar.activation(dumm, trimask[0:1, 0, 0:1], AF.Exp)

            # biases: b_q/b_k as [128,1] per-partition columns
            bqk = const.tile([128, 2], dt.float32)
            nc.sync.dma_start(
                out=bqk, in_=b_qkv[0:256].rearrange("(j p) -> p j", p=128)
            )

            # w_qkv rounded to fp32r (first DVE op: q/k path is the head of
            # the pipeline)
            w_sb = const.tile([128, 3 * C], dt.float32)
            nc.sync.dma_start(out=w_sb, in_=w_qkv[:, :])
            w_r = const.tile([128, 3 * C], dt.float32r)
            nc.vector.tensor_copy(w_r, w_sb)

            # persistent activations
            xT = big.tile([128, T], dt.float32r)       # [c, t]
            qkT = big.tile([128, 2, T], dt.float32r)   # [c, {q,k}, t]
            # vaug layout per tk-tile a: [128, 4 heads, 64]; head block =
            # [v_h (32) | 1.0 | zeros(31)]
            vaug = big.tile([128, NT, 4, 64], dt.bfloat16)

            def emit_x1(a):
                x_t = sb.tile([128, 128], dt.float32, tag="xin")
                nc.sync.dma_start(out=x_t, in_=x[128 * a:128 * (a + 1), :])
                p_tr = ps_misc.tile([128, 128], dt.float32, tag="misc")
                nc.tensor.transpose(p_tr, x_t, ident)
                nc.vector.tensor_copy(xT[:, 128 * a:128 * (a + 1)], p_tr)

            def emit_qk1(g, ch):
                p_qk = ps_misc.tile([128, TQ], dt.float32, tag="misc")
                nc.tensor.matmul(
                    p_qk,
                    w_r[:, 128 * ch:128 * (ch + 1)],
                    xT[:, TQ * g:TQ * (g + 1)],
                    start=True, stop=True,
                )
                nc.vector.tensor_scalar_add(
                    qkT[:, ch, TQ * g:TQ * (g + 1)], p_qk, bqk[:, ch:ch + 1]
                )

            def emit_v1(a):
                p_v = ps_misc.tile([128, 128], dt.float32, tag="misc")
                nc.tensor.matmul(
                    p_v,
                    xT[:, 128 * a:128 * (a + 1)],
                    w_r[:, 256:384],
                    start=True, stop=True,
                )
                nc.vector.tensor_add(
                    vaug[:, a, :, 0:32],
                    p_v.rearrange("p (h d) -> p h d", h=4),
                    bvb.rearrange("p (h d) -> p h d", h=4),
                )

            def emit_xqk(g):
                for a in range(4 * g, 4 * g + 4):
                    emit_x1(a)
                emit_qk1(g, 0)
                emit_qk1(g, 1)

            def emit_v(g):
                for a in range(4 * g, 4 * g + 4):
                    emit_v1(a)

            emit_xqk(0)

            # ---------------- remaining constants ----------------
            # w_proj split into two "pair" tiles matching the pair layout of
            # the PV output (head A rows 0-31, denominator row 32, zeros,
            # head B rows 64-95, ...).  Rows 32-63/96-127 must be zero so the
            # r*(1/r)=1 rows and zero rows contribute nothing.
            wp_pair = []
            for pair in range(2):
                wp_sb = const.tile([128, C], dt.float32, name=f"wp_sb_{pair}")
                nc.vector.memset(wp_sb, 0.0)
                nc.sync.dma_start(
                    out=wp_sb[0:32, :], in_=w_proj[64 * pair:64 * pair + 32, :]
                )
                nc.sync.dma_start(
                    out=wp_sb[64:96, :], in_=w_proj[64 * pair + 32:64 * pair + 64, :]
                )
                wp_r = const.tile([128, C], dt.float32r, name=f"wp_r_{pair}")
                nc.vector.tensor_copy(wp_r, wp_sb)
                wp_pair.append(wp_r)

            # broadcast tiles for free-dim biases (b_v, b_proj): row vector in
            # one partition, K=1 matmul against ones -> [128, 128] all rows.
            brow = const.tile([1, 256], dt.float32)
            nc.sync.dma_start(out=brow[:, 0:128], in_=b_qkv[256:384][None, :])
            nc.sync.dma_start(out=brow[:, 128:256], in_=b_proj[:][None, :])
            brow_r = const.tile([1, 256], dt.float32r)
            nc.vector.tensor_copy(brow_r, brow)
            ones1_f = const.tile([1, 128], dt.float32)
            nc.vector.memset(ones1_f, 1.0)
            ones1 = const.tile([1, 128], dt.float32r)
            nc.vector.tensor_copy(ones1, ones1_f)
            onesf = const.tile([128, 64], dt.float32)
            nc.vector.memset(onesf, 1.0)
            p_b = ps_misc.tile([128, 256], dt.float32, tag="misc")
            nc.tensor.matmul(p_b, ones1, brow_r, start=True, stop=True)
            bvb = const.tile([128, 128], dt.float32)    # b_v broadcast
            bpb4 = const.tile([128, 4, 128], dt.float32)  # b_proj bcast x4
            nc.vector.tensor_copy(bvb, p_b[:, 0:128])
            for m in range(4):
                nc.vector.tensor_copy(bpb4[:, m, :], p_b[:, 128:256])

            nc.vector.memset(vaug, 0.0)
            nc.vector.memset(vaug[:, :, :, 32:33], 1.0)

            emit_v(0)
            emit_xqk(1)
            emit_v(1)

            # ---------------- attention pipeline ----------------
            p_os = {}
            ynorms_d = {}

            def emit_pv(pend):
                p_yp, jp, pairp, e_p, a_p, off_p = pend
                for ih in range(2):
                    nc.tensor.matmul(
                        p_yp[64 * ih:64 * (ih + 1), off_p:],
                        vaug[:, a_p, 2 * pairp + ih, :],
                        e_p[:, TQ * ih + off_p:TQ * (ih + 1)],
                        start=(a_p == 0), stop=(a_p == 4 * jp + 3),
                        tile_position=(0, 64 * ih),
                    )

            def emit_norm(p_y, j, pair):
                # reciprocal straight off the PV PSUM tile (only rows 32/96 =
                # denominators matter), broadcast those rows across
                # partitions 0-63 / 64-127, one elementwise mult.
                rrec = ysb.tile([128, TQ], dt.float32, tag="rrec",
                                name=f"rrec_{j}_{pair}")
                nc.vector.reciprocal(rrec, p_y)
                ynorm = ysb.tile([128, TQ], dt.float32r, tag="ynorm",
                                 name=f"ynorm_{j}_{pair}", bufs=3)
                if j == NJ - 1:
                    # tail path: broadcast via fp32 K=1 matmuls (PE is idle
                    # here; skips the DRAM round-trip latency).  TT needs one
                    # operand in SBUF, so evacuate p_y alongside.
                    ynum = ysb.tile([128, TQ], dt.float32, tag="ynum",
                                    name=f"ynum_{j}_{pair}")
                    nc.vector.tensor_copy(ynum, p_y)
                    p_rb = ps_misc.tile([128, TQ], dt.float32, tag="misc")
                    for half in range(2):
                        nc.tensor.matmul(
                            p_rb[64 * half:64 * (half + 1), :],
                            onesf[32 + 64 * half:33 + 64 * half, :],
                            rrec[32 + 64 * half:33 + 64 * half, :],
                            start=True, stop=True,
                            tile_position=(32 + 64 * half, 64 * half),
                        )
                    nc.vector.tensor_mul(ynorm, ynum, p_rb)
                else:
                    rb = ysb.tile([128, TQ], dt.float32, tag="rb",
                                  name=f"rb_{j}_{pair}")
                    for half in range(2):
                        nc.sync.dma_start(
                            out=rscr[j, pair, half][None, :],
                            in_=rrec[32 + 64 * half:33 + 64 * half, :],
                        )
                        src = rscr[j, pair, half]
                        nc.sync.dma_start(
                            out=rb[64 * half:64 * (half + 1), :],
                            in_=bass.AP(
                                tensor=src.tensor,
                                offset=src.offset,
                                ap=[[0, 64], [1, TQ]],
                            ),
                        )
                    nc.vector.tensor_mul(ynorm, p_y, rb)
                return ynorm

            def emit_proj_m(j, m):
                p_o = ps_misc.tile([128, 128], dt.float32, tag="misc")
                for pr in range(2):
                    nc.tensor.matmul(
                        p_o,
                        ynorms_d[j][pr][:, 128 * m:128 * (m + 1)],
                        wp_pair[pr],
                        start=(pr == 0), stop=(pr == 1),
                    )
                o_t = sb.tile([128, 128], dt.float32, tag="out")
                nc.vector.tensor_add(o_t, p_o, bpb4[:, 0, :])
                t0 = TQ * j + 128 * m
                nc.sync.dma_start(out=y[t0:t0 + 128, :], in_=o_t)

            # flat software pipeline over all (j, pair, a) tiles: the PV
            # matmuls trail the S/exp stream by one item so the PE queue
            # always has independent S work ahead of each exp dependency;
            # group-boundary work (normalization, projection halves, QKV for
            # j+2) lands behind the next group's S matmuls.
            items = [
                (j, pair, a)
                for j in range(NJ)
                for pair in range(2)
                for a in range(4 * j + 4)
            ]
            p_ys = {}
            pend = None

            from collections import deque
            bg = deque()

            def flush(pend):
                emit_pv(pend)
                _, jp, pairp, _, a_p, _ = pend
                if a_p == 4 * jp + 3:  # group (jp, pairp) complete
                    ynorm = emit_norm(p_ys.pop((jp, pairp)), jp, pairp)
                    ynorms_d.setdefault(jp, []).append(ynorm)
                    if pairp == 1:
                        if jp + 2 < NJ:
                            g = jp + 2
                            for aa in range(4 * g, 4 * g + 4):
                                bg.append(lambda aa=aa: emit_x1(aa))
                            bg.append(lambda g=g: emit_qk1(g, 0))
                            bg.append(lambda g=g: emit_qk1(g, 1))
                            for aa in range(4 * g, 4 * g + 4):
                                bg.append(lambda aa=aa: emit_v1(aa))
                        for m in range(TQ // 128):
                            bg.append(lambda jp=jp, m=m: emit_proj_m(jp, m))
                if bg:
                    bg.popleft()()

            for j, pair, a in items:
                if a == 0:
                    p_ys[(j, pair)] = ps_y.tile(
                        [128, TQ], dt.float32, tag="py", name=f"p_y_{j}_{pair}"
                    )
                p_y = p_ys[(j, pair)]
                r = a - 4 * j
                off = 128 * r if r > 0 else 0
                p_s = ps_s.tile([128, 1024], dt.float32, tag="s")
                for ih, h in enumerate((2 * pair, 2 * pair + 1)):
                    nc.tensor.matmul(
                        p_s[:, TQ * ih + off:TQ * (ih + 1)],
                        qkT[32 * h:32 * (h + 1), 1, 128 * a:128 * (a + 1)],
                        qkT[32 * h:32 * (h + 1), 0, TQ * j + off:TQ * (j + 1)],
                        start=True, stop=True,
                        tile_position=(32 * h, 0),
                    )
                e_t = esb.tile([128, 1024], dt.bfloat16, tag="e")
                nc.scalar.activation(
                    e_t.rearrange("p (i f) -> p i f", i=2)[:, :, off:],
                    p_s.rearrange("p (i f) -> p i f", i=2)[:, :, off:],
                    AF.Exp,
                    scale=float(SCALE),
                )
                if r >= 0:
                    # diag tile: zero the causal triangle of the exp'd tile
                    # (tk > tq, i.e. partition p > local col f) on GPSIMD so
                    # the DVE stays off the S->exp->PV critical path
                    sel = e_t.rearrange("p (i f) -> p i f", i=2)[:, :, off:off + 128]
                    nc.gpsimd.affine_select(
                        out=sel,
                        in_=sel,
                        compare_op=mybir.AluOpType.is_ge,
                        fill=0.0,
                        base=0,
                        pattern=[[0, 2], [1, 128]],
                        channel_multiplier=-1,
                    )
                if pend is not None:
                    flush(pend)
                pend = (p_y, j, pair, e_t, a, off)
            flush(pend)
            while bg:
                bg.popleft()()

    nc.compile()
    return nc


def _get_nc():
    if "nc" not in _cache:
        _cache["nc"] = _build()
    return _cache["nc"]


def run(inputs, trace=False):
    from concourse.bass_utils import run_bass_kernel_spmd

    nc = _get_nc()
    x = np.asarray(inputs["x"], dtype=np.float32)
    w_qkv = np.ascontiguousarray(np.asarray(inputs["w_qkv"], dtype=np.float32))
    b_qkv = np.ascontiguousarray(np.asarray(inputs["b_qkv"], dtype=np.float32))
    w_proj = np.ascontiguousarray(np.asarray(inputs["w_proj"], dtype=np.float32))
    b_proj = np.ascontiguousarray(np.asarray(inputs["b_proj"], dtype=np.float32))
    in_maps = [
        {
            "x": np.ascontiguousarray(x[b]),
            "w_qkv": w_qkv,
            "b_qkv": b_qkv,
            "w_proj": w_proj,
            "b_proj": b_proj,
        }
        for b in range(N_CORES)
    ]
    res = run_bass_kernel_spmd(
        nc, in_maps, core_ids=list(range(N_CORES)), trace=trace
    )
    out = np.stack([res.results[b]["y"] for b in range(N_CORES)], axis=0)
    return out, res


def kernel(**inputs) -> np.ndarray:
    out, _ = run(inputs, trace=False)
    return out


# revision 33
# speedup vs baseline: 1.1022x; 1.0193x over previous
"""Causal self-attention (B=8, T=2048, C=128, H=4, D=32) on 8 trn2 NeuronCores.

Sharding: data-parallel over batch — core b handles batch element b.

Per-core algorithm (PE matmuls in fp32r = full-rate rounded fp32, except the
PV stage in bf16 so its head pair can col-tile across the PE array):
  xT = transpose(x)                      # PE transposes, [C, T]
  qT, kT = (x @ Wq|k + b)^T              # weights stationary, out [C,T] chunks
  v   = x @ Wv + bv                      # natural [T, C], packed into vaug
  vaug[tk-tile a] = [v_h | 1 | 0...]     # [128, 64] per head: the ones column
                                         # accumulates the softmax denominator
  flat software pipeline over (tq-block j, head pair, tk-tile a <= 4j+3):
      S^T[tk,tq] = kT_h.T @ qT_h         # K=32 row-packed pairs, PSUM [128,1024]
      (diag a: += lower-tri -30000 mask)
      E = exp(S * 1/sqrt(32))            # ACT, fused scale, bf16 out
      psum_y += vaug_a.T @ E             # col-packed pairs, M=64; row 32 = sum E
  per (j, pair): reciprocal of denominator rows, DMA broadcast via DRAM
  scratch, one multiply -> ynorm; projection accumulated per pair into one
  [128, 512] PSUM tile; one bias add + one DMA out per j.
"""

import sys

sys.path.insert(0, "/opt/trn_rl_repo")

import numpy as np

B, T, C = 8, 2048, 128
H, D = 4, 32
N_CORES = 8
TQ = 512          # tq block
NT = T // 128     # 16 tk tiles
NJ = T // TQ      # 4 tq blocks
SCALE = 1.0 / np.sqrt(D)
MASKVAL = -30000.0

_cache = {}


def _build():
    import concourse.bass as bass
    import concourse.mybir as mybir
    import concourse.tile as tile
    from concourse import bacc
    from concourse.masks import make_identity

    dt = mybir.dt
    AF = mybir.ActivationFunctionType
    nc = bacc.Bacc()

    x = nc.dram_tensor("x", [T, C], dt.float32, kind="ExternalInput")
    w_qkv = nc.dram_tensor("w_qkv", [C, 3 * C], dt.float32, kind="ExternalInput")
    b_qkv = nc.dram_tensor("b_qkv", [3 * C], dt.float32, kind="ExternalInput")
    w_proj = nc.dram_tensor("w_proj", [C, C], dt.float32, kind="ExternalInput")
    b_proj = nc.dram_tensor("b_proj", [C], dt.float32, kind="ExternalInput")
    y = nc.dram_tensor("y", [T, C], dt.float32, kind="ExternalOutput")
    # DRAM scratch for the denominator-reciprocal partition broadcast
    rscr = nc.dram_tensor("rscr", [NJ, 2, 2, TQ], dt.float32, kind="Internal")

    with tile.TileContext(nc) as tc:
        with (
            nc.allow_low_precision(reason="fp32r/bf16 matmuls; validated vs ref"),
            tc.tile_pool(name="const", bufs=1) as const,
            tc.tile_pool(name="big", bufs=1) as big,
            tc.tile_pool(name="sb", bufs=4) as sb,
            tc.tile_pool(name="esb", bufs=4) as esb,
            tc.tile_pool(name="ysb", bufs=3) as ysb,
            tc.tile_pool(name="ps_misc", bufs=2, space="PSUM") as ps_misc,
            tc.tile_pool(name="ps_s", bufs=2, space="PSUM") as ps_s,
            tc.tile_pool(name="ps_y", bufs=2, space="PSUM") as ps_y,
        ):
            # ---------------- critical-path constants ----------------
            ident = const.tile([128, 128], dt.float32)
            make_identity(nc, ident)

            # lower-triangle causal mask for S^T diag tiles, duplicated 2x so
            # both head-halves mask in one DVE op.  masked iff tk > tq i.e.
            # partition p > free f:  keep when (f - p) >= 0.
            trimask = const.tile([128, 2, 128], dt.float32)
            nc.gpsimd.memset(trimask, 0.0)
            for half in range(2):
                nc.gpsimd.affine_select(
                    out=trimask[:, half, :],
                    in_=trimask[:, half, :],
                    compare_op=mybir.AluOpType.is_ge,
                    fill=MASKVAL,
                    base=0,
                    pattern=[[1, 128]],
                    channel_multiplier=-1,
                )

            # dummy exp so the ACT table set loads while QKV runs
            dumm = const.tile([1, 1], dt.float32)
            nc.scalar.activation(dumm, trimask[0:1, 0, 0:1], AF.Exp)

            # biases: b_q/b_k as [128,1] per-partition columns
            bqk = const.tile([128, 2], dt.float32)
            nc.sync.dma_start(
                out=bqk, in_=b_qkv[0:256].rearrange("(j p) -> p j", p=128)
            )

            # w_qkv rounded to fp32r (first DVE op: q/k path is the head of
            # the pipeline)
            w_sb = const.tile([128, 3 * C], dt.float32)
            nc.sync.dma_start(out=w_sb, in_=w_qkv[:, :])
            w_r = const.tile([128, 3 * C], dt.float32r)
            nc.vector.tensor_copy(w_r, w_sb)

            # persistent activations
            xT = big.tile([128, T], dt.float32r)       # [c, t]
            qkT = big.tile([128, 2, T], dt.float32r)   # [c, {q,k}, t]
            # vaug layout per tk-tile a: [128, 4 heads, 64]; head block =
            # [v_h (32) | 1.0 | zeros(31)]
            vaug = big.tile([128, NT, 4, 64], dt.bfloat16)

            def emit_x1(a):
                x_t = sb.tile([128, 128], dt.float32, tag="xin")
                nc.sync.dma_start(out=x_t, in_=x[128 * a:128 * (a + 1), :])
                p_tr = ps_misc.tile([128, 128], dt.float32, tag="misc")
                nc.tensor.transpose(p_tr, x_t, ident)
                nc.vector.tensor_copy(xT[:, 128 * a:128 * (a + 1)], p_tr)

            def emit_qk1(g, ch):
                p_qk = ps_misc.tile([128, TQ], dt.float32, tag="misc")
                nc.tensor.matmul(
                    p_qk,
                    w_r[:, 128 * ch:128 * (ch + 1)],
                    xT[:, TQ * g:TQ * (g + 1)],
                    start=True, stop=True,
                )
                nc.vector.tensor_scalar_add(
                    qkT[:, ch, TQ * g:TQ * (g + 1)], p_qk, bqk[:, ch:ch + 1]
                )

            def emit_v1(a):
                p_v = ps_misc.tile([128, 128], dt.float32, tag="misc")
                nc.tensor.matmul(
                    p_v,
                    xT[:, 128 * a:128 * (a + 1)],
                    w_r[:, 256:384],
                    start=True, stop=True,
                )
                nc.vector.tensor_add(
                    vaug[:, a, :, 0:32],
                    p_v.rearrange("p (h d) -> p h d", h=4),
                    bvb.rearrange("p (h d) -> p h d", h=4),
                )

            def emit_xqk(g):
                for a in range(4 * g, 4 * g + 4):
                    emit_x1(a)
                emit_qk1(g, 0)
                emit_qk1(g, 1)

            def emit_v(g):
                for a in range(4 * g, 4 * g + 4):
                    emit_v1(a)

            emit_xqk(0)

            # ---------------- remaining constants ----------------
            # w_proj split into two "pair" tiles matching the pair layout of
            # the PV output (head A rows 0-31, denominator row 32, zeros,
            # head B rows 64-95, ...).  Rows 32-63/96-127 must be zero so the
            # r*(1/r)=1 rows and zero rows contribute nothing.
            wp_pair = []
            for pair in range(2):
                wp_sb = const.tile([128, C], dt.float32, name=f"wp_sb_{pair}")
                nc.vector.memset(wp_sb, 0.0)
                nc.sync.dma_start(
                    out=wp_sb[0:32, :], in_=w_proj[64 * pair:64 * pair + 32, :]
                )
                nc.sync.dma_start(
                    out=wp_sb[64:96, :], in_=w_proj[64 * pair + 32:64 * pair + 64, :]
                )
                wp_r = const.tile([128, C], dt.float32r, name=f"wp_r_{pair}")
                nc.vector.tensor_copy(wp_r, wp_sb)
                wp_pair.append(wp_r)

            # broadcast tiles for free-dim biases (b_v, b_proj): row vector in
            # one partition, K=1 matmul against ones -> [128, 128] all rows.
            brow = const.tile([1, 256], dt.float32)
            nc.sync.dma_start(out=brow[:, 0:128], in_=b_qkv[256:384][None, :])
            nc.sync.dma_start(out=brow[:, 128:256], in_=b_proj[:][None, :])
            brow_r = const.tile([1, 256], dt.float32r)
            nc.vector.tensor_copy(brow_r, brow)
            ones1_f = const.tile([1, 128], dt.float32)
            nc.vector.memset(ones1_f, 1.0)
            ones1 = const.tile([1, 128], dt.float32r)
            nc.vector.tensor_copy(ones1, ones1_f)
            onesf = const.tile([128, 64], dt.float32)
            nc.vector.memset(onesf, 1.0)
            p_b = ps_misc.tile([128, 256], dt.float32, tag="misc")
            nc.tensor.matmul(p_b, ones1, brow_r, start=True, stop=True)
            bvb = const.tile([128, 128], dt.float32)    # b_v broadcast
            bpb4 = const.tile([128, 4, 128], dt.float32)  # b_proj bcast x4
            nc.vector.tensor_copy(bvb, p_b[:, 0:128])
            for m in range(4):
                nc.vector.tensor_copy(bpb4[:, m, :], p_b[:, 128:256])

            nc.gpsimd.memset(vaug, 0.0)
            nc.gpsimd.memset(vaug[:, :, :, 32:33], 1.0)

            emit_v(0)
            emit_xqk(1)
            emit_v(1)

            # ---------------- attention pipeline ----------------
            p_os = {}
            ynorms_d = {}

            def emit_pv(pend):
                p_yp, jp, pairp, e_p, a_p, off_p = pend
                for ih in range(2):
                    nc.tensor.matmul(
                        p_yp[64 * ih:64 * (ih + 1), off_p:],
                        vaug[:, a_p, 2 * pairp + ih, :],
                        e_p[:, TQ * ih + off_p:TQ * (ih + 1)],
                        start=(a_p == 0), stop=(a_p == 4 * jp + 3),
                        tile_position=(0, 64 * ih),
                    )

            def emit_norm(p_y, j, pair):
                # reciprocal straight off the PV PSUM tile (only rows 32/96 =
                # denominators matter), broadcast those rows across
                # partitions 0-63 / 64-127, one elementwise mult.
                rrec = ysb.tile([128, TQ], dt.float32, tag="rrec",
                                name=f"rrec_{j}_{pair}")
                nc.vector.reciprocal(rrec, p_y)
                ynorm = ysb.tile([128, TQ], dt.float32r, tag="ynorm",
                                 name=f"ynorm_{j}_{pair}", bufs=3)
                if j == NJ - 1:
                    # tail path: broadcast via fp32 K=1 matmuls (PE is idle
                    # here; skips the DRAM round-trip latency).  TT needs one
                    # operand in SBUF, so evacuate p_y alongside.
                    ynum = ysb.tile([128, TQ], dt.float32, tag="ynum",
                                    name=f"ynum_{j}_{pair}")
                    nc.vector.tensor_copy(ynum, p_y)
                    p_rb = ps_misc.tile([128, TQ], dt.float32, tag="misc")
                    for half in range(2):
                        nc.tensor.matmul(
                            p_rb[64 * half:64 * (half + 1), :],
                            onesf[32 + 64 * half:33 + 64 * half, :],
                            rrec[32 + 64 * half:33 + 64 * half, :],
                            start=True, stop=True,
                            tile_position=(32 + 64 * half, 64 * half),
                        )
                    nc.vector.tensor_mul(ynorm, ynum, p_rb)
                else:
                    rb = ysb.tile([128, TQ], dt.float32, tag="rb",
                                  name=f"rb_{j}_{pair}")
                    for half in range(2):
                        nc.sync.dma_start(
                            out=rscr[j, pair, half][None, :],
                            in_=rrec[32 + 64 * half:33 + 64 * half, :],
                        )
                        src = rscr[j, pair, half]
                        nc.sync.dma_start(
                            out=rb[64 * half:64 * (half + 1), :],
                            in_=bass.AP(
                                tensor=src.tensor,
                                offset=src.offset,
                                ap=[[0, 64], [1, TQ]],
                            ),
                        )
                    nc.vector.tensor_mul(ynorm, p_y, rb)
                return ynorm

            def emit_proj_m(j, m):
                p_o = ps_misc.tile([128, 128], dt.float32, tag="misc")
                for pr in range(2):
                    nc.tensor.matmul(
                        p_o,
                        ynorms_d[j][pr][:, 128 * m:128 * (m + 1)],
                        wp_pair[pr],
                        start=(pr == 0), stop=(pr == 1),
                    )
                o_t = sb.tile([128, 128], dt.float32, tag="out")
                nc.vector.tensor_add(o_t, p_o, bpb4[:, 0, :])
                t0 = TQ * j + 128 * m
                nc.sync.dma_start(out=y[t0:t0 + 128, :], in_=o_t)

            # flat software pipeline over all (j, pair, a) tiles: the PV
            # matmuls trail the S/exp stream by one item so the PE queue
            # always has independent S work ahead of each exp dependency;
            # group-boundary work (normalization, projection halves, QKV for
            # j+2) lands behind the next group's S matmuls.
            items = [
                (j, pair, a)
                for j in range(NJ)
                for pair in range(2)
                for a in range(4 * j + 4)
            ]
            p_ys = {}
            pend = None

            from collections import deque
            bg = deque()

            def flush(pend):
                emit_pv(pend)
                _, jp, pairp, _, a_p, _ = pend
                if a_p == 4 * jp + 3:  # group (jp, pairp) complete
                    ynorm = emit_norm(p_ys.pop((jp, pairp)), jp, pairp)
                    ynorms_d.setdefault(jp, []).append(ynorm)
                    if pairp == 1:
                        if jp + 2 < NJ:
                            g = jp + 2
                            for aa in range(4 * g, 4 * g + 4):
                                bg.append(lambda aa=aa: emit_x1(aa))
                            bg.append(lambda g=g: emit_qk1(g, 0))
                            bg.append(lambda g=g: emit_qk1(g, 1))
                            for aa in range(4 * g, 4 * g + 4):
                                bg.append(lambda aa=aa: emit_v1(aa))
                        for m in range(TQ // 128):
                            bg.append(lambda jp=jp, m=m: emit_proj_m(jp, m))
                if bg:
                    bg.popleft()()

            for j, pair, a in items:
                if a == 0:
                    p_ys[(j, pair)] = ps_y.tile(
                        [128, TQ], dt.float32, tag="py", name=f"p_y_{j}_{pair}"
                    )
                p_y = p_ys[(j, pair)]
                r = a - 4 * j
                off = 128 * r if r > 0 else 0
                p_s = ps_s.tile([128, 1024], dt.float32, tag="s")
                for ih, h in enumerate((2 * pair, 2 * pair + 1)):
                    nc.tensor.matmul(
                        p_s[:, TQ * ih + off:TQ * (ih + 1)],
                        qkT[32 * h:32 * (h + 1), 1, 128 * a:128 * (a + 1)],
                        qkT[32 * h:32 * (h + 1), 0, TQ * j + off:TQ * (j + 1)],
                        start=True, stop=True,
                        tile_position=(32 * h, 0),
                    )
                e_t = esb.tile([128, 1024], dt.bfloat16, tag="e")
                nc.scalar.activation(
                    e_t.rearrange("p (i f) -> p i f", i=2)[:, :, off:],
                    p_s.rearrange("p (i f) -> p i f", i=2)[:, :, off:],
                    AF.Exp,
                    scale=float(SCALE),
                )
                if r >= 0:
                    # diag tile: zero the causal triangle of the exp'd tile
                    # (tk > tq, i.e. partition p > local col f) on GPSIMD so
                    # the DVE stays off the S->exp->PV critical path
                    sel = e_t.rearrange("p (i f) -> p i f", i=2)[:, :, off:off + 128]
                    nc.gpsimd.affine_select(
                        out=sel,
                        in_=sel,
                        compare_op=mybir.AluOpType.is_ge,
                        fill=0.0,
                        base=0,
                        pattern=[[0, 2], [1, 128]],
                        channel_multiplier=-1,
                    )
                if pend is not None:
                    flush(pend)
                pend = (p_y, j, pair, e_t, a, off)
            flush(pend)
            while bg:
                bg.popleft()()

    nc.compile()
    return nc


def _get_nc():
    if "nc" not in _cache:
        _cache["nc"] = _build()
    return _cache["nc"]


def run(inputs, trace=False):
    from concourse.bass_utils import run_bass_kernel_spmd

    nc = _get_nc()
    x = np.asarray(inputs["x"], dtype=np.float32)
    w_qkv = np.ascontiguousarray(np.asarray(inputs["w_qkv"], dtype=np.float32))
    b_qkv = np.ascontiguousarray(np.asarray(inputs["b_qkv"], dtype=np.float32))
    w_proj = np.ascontiguousarray(np.asarray(inputs["w_proj"], dtype=np.float32))
    b_proj = np.ascontiguousarray(np.asarray(inputs["b_proj"], dtype=np.float32))
    in_maps = [
        {
            "x": np.ascontiguousarray(x[b]),
            "w_qkv": w_qkv,
            "b_qkv": b_qkv,
            "w_proj": w_proj,
            "b_proj": b_proj,
        }
        for b in range(N_CORES)
    ]
    res = run_bass_kernel_spmd(
        nc, in_maps, core_ids=list(range(N_CORES)), trace=trace
    )
    out = np.stack([res.results[b]["y"] for b in range(N_CORES)], axis=0)
    return out, res


def kernel(**inputs) -> np.ndarray:
    out, _ = run(inputs, trace=False)
    return out
